# revision 2
# baseline (speedup 1.0000x reference)
"""KNN classification kernel for Trainium2 (Bass/Tile), 8-core SPMD.

Problem: 1-query KNN over train_data [500000, 256] f32, K=3, 10 classes.
    distances = ||x - train_data||_2  -> top-3 smallest -> mode of targets.

Strategy (row-sharded):
  - Shard train_data row-wise across 8 cores (62500 rows each).
  - Each core streams its 64MB shard through SBUF in 2MB super-tiles
    ([128 partitions x 16 row-groups x 256], row r = t*128 + p), computing
    squared distances:
        DVE: diff = tile - broadcast(x)            (1 pass)
        ACT: Square + accum_out per 256-segment    (14/16 segments)
        DVE: tensor_tensor_reduce per segment      ( 2/16 segments)
    accumulating a [128, 489] buffer of squared distances.
  - Negate, then vector.max_with_indices gives the top-8 smallest
    distances + column indices per partition (top-3 global candidates of a
    core are always within its per-partition top-8).
  - Host merges 8 cores x 128 partitions x 8 candidates (tiny), picks the
    global top-3 by (distance, index) and computes the mode with
    smallest-value tie-break (torch .mode semantics).

The kernel is memory-bound: per-core roofline = 64MB / ~358 GB/s ~= 180us.
"""

import sys

import numpy as np

for _p in ("/opt/trn_rl_repo",):
    if _p not in sys.path:
        sys.path.insert(0, _p)

import concourse.bacc as bacc
import concourse.mybir as mybir
from concourse import tile
from concourse.bass_utils import run_bass_kernel_spmd

N_TRAIN = 500000
D = 256
CORES = 8
K = 3
N_SHARD = N_TRAIN // CORES  # 62500
P = 128
ST_ROWS = 2048  # rows per super-tile -> [128, 4096] = 2MB DMAs
BIG = 1.0e30
FP32 = mybir.dt.float32
U32 = mybir.dt.uint32
# Of the row-groups (256-element segments) in a super-tile, this fraction is
# reduced on DVE via tensor_tensor_reduce; the rest on ACT via Square+accum.
DVE_SEG_FRAC = 2 / 16


def build_knn(tc, x_ap, td_ap, vals_ap, idx_ap, n_shard):
    """Emit the per-core KNN distance + top-8 program under TileContext."""
    nc = tc.nc
    n_cols = -(-n_shard // P)  # columns of the distance buffer
    st_free = ST_ROWS * D // P  # 4096

    with (
        tc.tile_pool(name="xbp", bufs=1) as xb_pool,
        tc.tile_pool(name="dbp", bufs=1) as d_pool,
        tc.tile_pool(name="inp", bufs=4) as in_pool,
        tc.tile_pool(name="dfp", bufs=3) as diff_pool,
        tc.tile_pool(name="scp", bufs=6) as scr_pool,
        tc.tile_pool(name="outp", bufs=1) as out_pool,
    ):
        # x broadcast to [128, 4096] (repeated along partitions and 16x free)
        xb = xb_pool.tile([P, st_free], FP32)
        nc.sync.dma_start(out=xb[:, 0:D], in_=x_ap[None, :].partition_broadcast(P))
        w = D
        while w < st_free:
            nc.vector.tensor_copy(out=xb[:, w : 2 * w], in_=xb[:, 0:w])
            w *= 2

        # squared-distance accumulator; D[p, t] = ||row(t*128+p) - x||^2
        dpos = d_pool.tile([P, n_cols], FP32)
        nc.vector.memset(dpos[:], BIG)

        col, r = 0, 0
        while r < n_shard:
            rows = min(ST_ROWS, n_shard - r)
            a = rows // P  # full 128-row groups
            rem = rows - a * P  # leftover rows (<128), only on last chunk
            if a:
                wfree = a * D
                t_in = in_pool.tile([P, wfree], FP32, tag="tin")
                nc.sync.dma_start(
                    out=t_in[:].rearrange("p (a d) -> p a d", d=D),
                    in_=td_ap[r : r + a * P, :].rearrange("(a p) d -> p a d", p=P),
                )
                diff = diff_pool.tile([P, wfree], FP32, tag="diff")
                nc.vector.tensor_sub(diff[:], t_in[:], xb[:, 0:wfree])
                n_dve = int(round(a * DVE_SEG_FRAC))
                for s in range(a):
                    seg = diff[:, s * D : (s + 1) * D]
                    acc = dpos[:, col + s : col + s + 1]
                    scr = scr_pool.tile([P, D], FP32, tag="scr")
                    if s >= a - n_dve:
                        nc.vector.scalar_tensor_tensor(
                            out=scr[:],
                            in0=seg,
                            scalar=0.0,
                            in1=seg,
                            op0=mybir.AluOpType.bypass,
                            op1=mybir.AluOpType.mult,
                            accum_out=acc,
                        )
                    else:
                        nc.scalar.activation(
                            scr[:],
                            seg,
                            mybir.ActivationFunctionType.Square,
                            accum_out=acc,
                        )
                col += a
                r += a * P
            if rem:
                t_t = in_pool.tile([P, D], FP32, tag="tin_tail")
                nc.sync.dma_start(out=t_t[0:rem, :], in_=td_ap[r : r + rem, :])
                difft = diff_pool.tile([P, D], FP32, tag="diff_tail")
                nc.vector.tensor_sub(difft[0:rem, :], t_t[0:rem, :], xb[0:rem, 0:D])
                scrt = scr_pool.tile([P, D], FP32, tag="scr")
                nc.scalar.activation(
                    scrt[0:rem, :],
                    difft[0:rem, :],
                    mybir.ActivationFunctionType.Square,
                    accum_out=dpos[0:rem, col : col + 1],
                )
                col += 1
                r += rem
        assert col == n_cols, (col, n_cols)

        # top-8 smallest distances per partition = top-8 largest of -dpos
        dneg = out_pool.tile([P, n_cols], FP32)
        nc.scalar.mul(dneg[:], dpos[:], -1.0)
        valt = out_pool.tile([P, 8], FP32)
        idxt = out_pool.tile([P, 8], U32)
        nc.vector.max_with_indices(valt[:], idxt[:], dneg[:])
        nc.sync.dma_start(out=vals_ap[:, :], in_=valt[:])
        nc.sync.dma_start(out=idx_ap[:, :], in_=idxt[:])


_PROGRAM_CACHE = {}


def get_program(n_shard=N_SHARD):
    if n_shard not in _PROGRAM_CACHE:
        nc = bacc.Bacc(
            "TRN2", target_bir_lowering=False, debug=False, num_devices=CORES
        )
        x_t = nc.dram_tensor("x", [D], FP32, kind="ExternalInput")
        td_t = nc.dram_tensor("td", [n_shard, D], FP32, kind="ExternalInput")
        vals_t = nc.dram_tensor("out_vals", [P, 8], FP32, kind="ExternalOutput")
        idx_t = nc.dram_tensor("out_idx", [P, 8], U32, kind="ExternalOutput")
        with tile.TileContext(nc) as tc:
            build_knn(tc, x_t.ap(), td_t.ap(), vals_t.ap(), idx_t.ap(), n_shard)
        nc.compile()
        _PROGRAM_CACHE[n_shard] = nc
    return _PROGRAM_CACHE[n_shard]


def run_device(in_maps, trace=False, trace_cores=None):
    nc = get_program()
    return run_bass_kernel_spmd(
        nc, in_maps, list(range(CORES)), trace=trace, trace_cores=trace_cores
    )


def make_in_maps(x, train_data):
    x = np.ascontiguousarray(np.asarray(x, dtype=np.float32))
    train_data = np.asarray(train_data, dtype=np.float32)
    return [
        {
            "x": x,
            "td": np.ascontiguousarray(train_data[c * N_SHARD : (c + 1) * N_SHARD]),
        }
        for c in range(CORES)
    ]


def merge_results(results, train_targets):
    """Merge per-core top-8-per-partition candidates into the predicted class."""
    ds, gs = [], []
    p_idx = np.arange(P, dtype=np.int64)[:, None]
    for c in range(CORES):
        v = np.asarray(results[c]["out_vals"], dtype=np.float64)
        ix = np.asarray(results[c]["out_idx"], dtype=np.int64)
        d2 = -v  # squared distances
        g = c * N_SHARD + ix * P + p_idx
        valid = d2 < BIG / 2
        ds.append(d2[valid])
        gs.append(g[valid])
    d = np.concatenate(ds)
    gi = np.concatenate(gs)
    order = np.lexsort((gi, d))  # by distance asc, then index asc (top_k ties)
    top = gi[order[:K]]
    knn_t = np.asarray(train_targets)[top]
    # torch .mode(): most frequent value, smallest value on ties
    counts = (knn_t[:, None] == knn_t[None, :]).sum(axis=1)
    sentinel = np.iinfo(knn_t.dtype).max
    cands = np.where(counts == counts.max(), knn_t, sentinel)
    return cands.min()


def kernel(x, train_data, train_targets):
    train_targets = np.asarray(train_targets)
    in_maps = make_in_maps(x, train_data)
    results = run_device(in_maps).results
    pred = merge_results(results, train_targets)
    return np.array(pred, dtype=train_targets.dtype)
